# revision 22
# baseline (speedup 1.0000x reference)
"""Trainium2 Bass kernel for nn_EventTemplateBank (batched 1-D template-bank conv).

Math: score[b,t,e] = sum_{f,l} delayed[e,f,l] * x[b, t+40-l, f] / (L*F),
with delayed = delay-shifted templates (zero fill) and x zero-padded.

Device formulation (per core, data-parallel over batch):
  Feature-packed Toeplitz: contraction partitions hold (feature, tap-chunk)
  pairs, K = 6*21 = 126, so one matmul contracts all 6 features over a
  21-tap window. Each rhs column covers Q=24 output positions; the 103-tap
  span (24+79) is accumulated over NCH=5 chunks. Outputs (d in [0,24),
  e in [0,16)) form NM=3 M-tiles of 128.
    X[(f,c), j, col=n] = x[b, 24n + 21j + c - 39, f]      (bf16, host im2col)
    W[(f,c), j, m, (dd,e)] = delayed[e, f, 8m+dd+79-21j-c] / 480
    ps[m][(dd,e), n] += sum_j W[:, j, m].T @ X[:, j, n-block]
  163,920 column-passes/core (vs 196,704 for the single-feature window) =
  68.3 us PE floor; 22.6 MB DMA/core ~= 71 us: balanced rooflines.
  PSUM f32 -> bf16 evac; host upcasts and re-permutes (t = 24n + 8m + dd).
"""

import numpy as np
import ml_dtypes

import concourse.mybir as mybir
from concourse import bacc
from concourse.bass_utils import run_bass_kernel_spmd
from concourse.tile import TileContext

BF16 = ml_dtypes.bfloat16

# Problem shapes (hardcoded per contract)
B, S, F = 64, 32768, 6
E, L = 16, 80
MAX_DELAY = 10

NCORES = 8
BPC = B // NCORES          # batches per core
Q = 24                     # output positions per rhs column
C = 21                     # taps per feature per chunk
NCH = 5                    # accumulation chunks (cover 24+79 = 103 <= 105 taps)
KP = F * C                 # 126 contraction partitions (chunks 0-3)
C4 = 19                    # chunk 4 only needs taps phi = 45..63 -> c in [0,19)
KP4 = F * C4               # 114 contraction partitions for chunk 4
NM = 3                     # M-tiles: (dd in [0,8)) x (e in [0,16)) per tile
PADL = 39                  # chunk sample index = 24n + 21j + c - 39
NCOLB = (S + Q - 1) // Q   # 1366 columns per batch
CTOT = BPC * NCOLB         # 10928 columns per core
# Small blocks first so the PE starts after ~0.15MB of input and ramps its
# p-state on cheap blocks; tiny trailing blocks keep the final
# matmul->cast->store drain chain short.
BLOCKS = [128, 128, 256] + [512] * 20 + [176]
assert sum(BLOCKS) == CTOT
NBLK = len(BLOCKS)
BOFF = [sum(BLOCKS[:i]) for i in range(NBLK)]
N_JOUTER = 3               # leading blocks run j-outer (gate on per-j W pieces)

LAST_RESULT = None         # BassKernelResults of the most recent run (for profiling)


def _build_weights(templates: np.ndarray, onset_delays: np.ndarray) -> np.ndarray:
    """W[(f,c), j, m, 16dd+e] = delayed[e, f, 8m+dd+79-21j-c] / (L*F)."""
    d = np.round(np.clip(onset_delays, -MAX_DELAY, MAX_DELAY)).astype(np.int64)
    idx = np.arange(L)
    src = idx[None, None, :] - d[:, :, None]                 # (E,F,L)
    valid = (src >= 0) & (src < L)
    delayed = np.take_along_axis(templates, np.clip(src, 0, L - 1), axis=2)
    delayed = np.where(valid, delayed, 0.0).astype(np.float32) / float(L * F)

    f_i = np.arange(F)[:, None, None, None, None]
    c_i = np.arange(C)[None, :, None, None, None]
    j_i = np.arange(NCH)[None, None, :, None, None]
    dd_i = np.arange(8)[None, None, None, :, None]
    m_i = np.arange(NM)[None, None, None, None, :]
    l = (8 * m_i + dd_i) + 79 - 21 * j_i - c_i               # (F,C,NCH,8,NM)
    ok = (l >= 0) & (l < L)
    g = delayed[:, f_i, np.clip(l, 0, L - 1)]                # (E,F,C,NCH,8,NM)
    g = np.where(ok[None], g, 0.0)
    # -> [(f,c), j, m, dd, e]
    W = g.transpose(1, 2, 3, 5, 4, 0).reshape(KP, NCH, NM, 8 * E)
    Wmain = np.ascontiguousarray(W[:, :NCH - 1]).astype(BF16)    # (KP, 4, NM, 128)
    W4 = W[:, NCH - 1].reshape(F, C, NM, 8 * E)[:, :C4]          # rows (f, c<19)
    W4 = np.ascontiguousarray(W4.reshape(KP4, NM, 8 * E)).astype(BF16)
    return Wmain, W4


def _build_xsc(x: np.ndarray) -> np.ndarray:
    """Xsc[core, (f,c), :] = block-major concat of [NCH, n_blk] chunk rows:
    chunk j of column col = 1366*b_local + n reads x[b, 24n + 21j + c - 39, f]."""
    need = Q * (NCOLB - 1) + 21 * (NCH - 1) + C
    xpad = np.zeros((B, PADL + need, F), dtype=np.float32)
    xpad[:, PADL:PADL + S, :] = x
    sb, st, sf = xpad.strides
    # V[b, (f,c), j, n] = xpad[b, 24n + 21j + c, f]
    V = np.lib.stride_tricks.as_strided(
        xpad, shape=(B, F, C, NCH, NCOLB), strides=(sb, sf, st, 21 * st, Q * st)
    )
    V16 = V.astype(BF16)                                   # (B, F, C, NCH, NCOLB)
    Xc = np.empty((NCORES, KP, NCH - 1, CTOT), dtype=BF16)
    X4c = np.empty((NCORES, KP4, CTOT), dtype=BF16)
    for b in range(B):
        core, i = divmod(b, BPC)
        sl = slice(i * NCOLB, (i + 1) * NCOLB)
        Xc[core, :, :, sl] = V16[b, :, :, :NCH - 1].reshape(KP, NCH - 1, NCOLB)
        X4c[core, :, sl] = V16[b, :, :C4, NCH - 1].reshape(KP4, NCOLB)
    out = np.empty((NCORES, KP, (NCH - 1) * CTOT), dtype=BF16)
    NJ = NCH - 1
    for off, n in zip(BOFF, BLOCKS):
        out[:, :, NJ * off:NJ * (off + n)] = (
            Xc[:, :, :, off:off + n].reshape(NCORES, KP, NJ * n)
        )
    return np.ascontiguousarray(out), np.ascontiguousarray(X4c)


def _build_program():
    f32 = mybir.dt.float32
    bf16 = mybir.dt.bfloat16
    NJ = NCH - 1
    nc = bacc.Bacc("TRN2", target_bir_lowering=False, debug=False)
    xsc = nc.dram_tensor("xsc", [KP, NJ * CTOT], bf16, kind="ExternalInput")
    x4 = nc.dram_tensor("x4", [KP4, CTOT], bf16, kind="ExternalInput")
    w = nc.dram_tensor("w", [KP, NJ, NM, 128], bf16, kind="ExternalInput")
    w4 = nc.dram_tensor("w4", [KP4, NM, 128], bf16, kind="ExternalInput")
    osc = nc.dram_tensor("osc", [128, NM * CTOT], bf16, kind="ExternalOutput")

    with TileContext(nc) as tc:
        with (
            tc.tile_pool(name="wp", bufs=1) as wp,
            tc.tile_pool(name="xp", bufs=6) as xp,
            tc.tile_pool(name="x4p", bufs=6) as x4p,
            tc.tile_pool(name="pp", bufs=8, space="PSUM") as pp,
            tc.tile_pool(name="op", bufs=6) as op,
        ):
            wt = wp.tile([KP, NJ * NM * 128], bf16)      # [(f,c), (j, m, col)]
            w4t = wp.tile([KP4, NM * 128], bf16)
            wr = w.rearrange("k j m n -> k (j m n)")
            w4r = w4.rearrange("k m n -> k (m n)")
            xtiles = {}

            def issue_w(j):
                if j < NJ:
                    sl = slice(j * NM * 128, (j + 1) * NM * 128)
                    nc.sync.dma_start(out=wt[:, sl], in_=wr[:, sl])
                else:
                    nc.sync.dma_start(out=w4t, in_=w4r)

            def issue_x(blk):
                off, n = BOFF[blk], BLOCKS[blk]
                xt = xp.tile([KP, NJ * n], bf16, tag="xt", name=f"xt_{blk}")
                nc.sync.dma_start(out=xt, in_=xsc[:, NJ * off:NJ * (off + n)])
                x4t = x4p.tile([KP4, n], bf16, tag="x4t", name=f"x4t_{blk}")
                nc.sync.dma_start(out=x4t, in_=x4[:, off:off + n])
                xtiles[blk] = (xt, x4t)

            def wslice(j, m):
                return wt[:, (j * NM + m) * 128:(j * NM + m + 1) * 128]

            def w4slice(m):
                return w4t[:, m * 128:(m + 1) * 128]

            # DMA order: first matmul gates on W(j0)+X0 (~0.25MB); later
            # pieces and x blocks stream in behind it.
            issue_w(0); issue_x(0)
            issue_w(1); issue_x(1)
            issue_w(2); issue_x(2)
            issue_w(3); issue_w(4)

            for blk in range(NBLK):
                off, n = BOFF[blk], BLOCKS[blk]
                if blk + 3 < NBLK:
                    issue_x(blk + 3)
                xt, x4t = xtiles.pop(blk)
                pss = [
                    pp.tile([128, n], f32, tag="ps", name=f"ps_{blk}_{m}")
                    for m in range(NM)
                ]
                ot = op.tile([128, NM * n], bf16, tag="ot", name=f"ot_{blk}")
                last = blk == NBLK - 1

                def mm(j, m, start, stop, skip=False, n=n, xt=xt, x4t=x4t, pss=pss):
                    if j < NJ:
                        lhsT, rhs = wslice(j, m), xt[:, j * n:(j + 1) * n]
                    else:
                        lhsT, rhs = w4slice(m), x4t
                    nc.tensor.matmul(pss[m], lhsT, rhs, start=start, stop=stop,
                                     skip_group_check=skip)

                def evac(m, n=n, pss=pss, ot=ot, off=off, last=last):
                    nc.vector.tensor_copy(out=ot[:, m * n:(m + 1) * n], in_=pss[m])
                    if last:
                        # stagger the final block's store per M-tile so the
                        # tail drain is one small DMA, not three
                        nc.sync.dma_start(
                            out=osc[:, NM * off + m * n:NM * off + (m + 1) * n],
                            in_=ot[:, m * n:(m + 1) * n],
                        )

                if blk < N_JOUTER:
                    # j-outer: each arriving W(j) piece feeds all 3 M-tiles.
                    for j in range(NCH):
                        for m in range(NM):
                            mm(j, m, start=(j == 0), stop=(j == NCH - 1), skip=True)
                    for m in range(NM):
                        evac(m)
                else:
                    # m-outer: M-tiles complete one after another, so PSUM
                    # evacuation staggers across the block.
                    for m in range(NM):
                        for j in range(NCH):
                            mm(j, m, start=(j == 0), stop=(j == NCH - 1))
                        evac(m)
                if not last:
                    nc.sync.dma_start(
                        out=osc[:, NM * off:NM * (off + n)], in_=ot
                    )
    nc.compile()   # bacc passes: split multi-waits (HW allows 1 wait/inst), DCE, reg alloc
    return nc


def kernel(x: np.ndarray, templates: np.ndarray, onset_delays: np.ndarray) -> np.ndarray:
    global LAST_RESULT
    x = np.ascontiguousarray(x, dtype=np.float32)
    templates = np.asarray(templates, dtype=np.float32)
    onset_delays = np.asarray(onset_delays, dtype=np.float32)

    Wm, W4 = _build_weights(templates, onset_delays)
    Xsc, X4c = _build_xsc(x)                              # (NCORES, KP, 4*CTOT), (NCORES, KP4, CTOT)

    nc = _build_program()
    in_maps = [{"xsc": Xsc[c], "x4": X4c[c], "w": Wm, "w4": W4} for c in range(NCORES)]
    res = run_bass_kernel_spmd(nc, in_maps, core_ids=list(range(NCORES)))
    LAST_RESULT = res

    osc = np.stack([r["osc"] for r in res.results], axis=0)   # (NCORES,128,NM*CTOT)
    osc = osc.astype(np.float32)
    O = np.empty((NCORES, 128, NM, CTOT), dtype=np.float32)
    for off, n in zip(BOFF, BLOCKS):
        O[:, :, :, off:off + n] = (
            osc[:, :, NM * off:NM * (off + n)].reshape(NCORES, 128, NM, n)
        )
    o = O.reshape(NCORES, 8, E, NM, BPC, NCOLB)           # c, dd, e, m, b, n
    o = o.transpose(0, 4, 5, 3, 1, 2)                      # c, b, n, m, dd, e
    o = np.ascontiguousarray(o).reshape(B, NCOLB * Q, E)[:, :S, :]
    o = np.ascontiguousarray(o)
    o[:, S - 1, :] = 0.0                                   # reference zero-pads last column
    return o


# revision 23
# speedup vs baseline: 1.0715x; 1.0715x over previous
"""Trainium2 Bass kernel for nn_EventTemplateBank (batched 1-D template-bank conv).

Math: score[b,t,e] = sum_{f,l} delayed[e,f,l] * x[b, t+40-l, f] / (L*F),
with delayed = delay-shifted templates (zero fill) and x zero-padded.

Device formulation (per core, data-parallel over batch):
  Feature-packed Toeplitz: contraction partitions hold (feature, tap-chunk)
  pairs, K = 6*21 = 126, so one matmul contracts all 6 features over a
  21-tap window. Each rhs column covers Q=24 output positions; the 103-tap
  span (24+79) is accumulated over NCH=5 chunks. Outputs (d in [0,24),
  e in [0,16)) form NM=3 M-tiles of 128.
    X[(f,c), j, col=n] = x[b, 24n + 21j + c - 39, f]      (bf16, host im2col)
    W[(f,c), j, m, (dd,e)] = delayed[e, f, 8m+dd+79-21j-c] / 480
    ps[m][(dd,e), n] += sum_j W[:, j, m].T @ X[:, j, n-block]
  163,920 column-passes/core (vs 196,704 for the single-feature window) =
  68.3 us PE floor; 22.6 MB DMA/core ~= 71 us: balanced rooflines.
  PSUM f32 -> bf16 evac; host upcasts and re-permutes (t = 24n + 8m + dd).
"""

import numpy as np
import ml_dtypes

import concourse.mybir as mybir
from concourse import bacc
from concourse.bass_utils import run_bass_kernel_spmd
from concourse.tile import TileContext

BF16 = ml_dtypes.bfloat16

# Problem shapes (hardcoded per contract)
B, S, F = 64, 32768, 6
E, L = 16, 80
MAX_DELAY = 10

NCORES = 8
BPC = B // NCORES          # batches per core
Q = 24                     # output positions per rhs column
C = 21                     # taps per feature per chunk
NCH = 5                    # accumulation chunks (cover 24+79 = 103 <= 105 taps)
KP = F * C                 # 126 contraction partitions
NM = 3                     # M-tiles: (dd in [0,8)) x (e in [0,16)) per tile
PADL = 39                  # chunk sample index = 24n + 21j + c - 39
NCOLB = (S + Q - 1) // Q   # 1366 columns per batch
CTOT = BPC * NCOLB         # 10928 columns per core
# Small blocks first so the PE starts after ~0.25MB of input and ramps its
# p-state on cheap blocks; small remainder last for a short drain chain.
BLOCKS = [128, 128, 256] + [512] * 20 + [176]
assert sum(BLOCKS) == CTOT
NBLK = len(BLOCKS)
BOFF = [sum(BLOCKS[:i]) for i in range(NBLK)]
N_JOUTER = 3               # leading blocks run j-outer (gate on per-j W pieces)

LAST_RESULT = None         # BassKernelResults of the most recent run (for profiling)


def _build_weights(templates: np.ndarray, onset_delays: np.ndarray) -> np.ndarray:
    """W[(f,c), j, m, 16dd+e] = delayed[e, f, 8m+dd+79-21j-c] / (L*F)."""
    d = np.round(np.clip(onset_delays, -MAX_DELAY, MAX_DELAY)).astype(np.int64)
    idx = np.arange(L)
    src = idx[None, None, :] - d[:, :, None]                 # (E,F,L)
    valid = (src >= 0) & (src < L)
    delayed = np.take_along_axis(templates, np.clip(src, 0, L - 1), axis=2)
    delayed = np.where(valid, delayed, 0.0).astype(np.float32) / float(L * F)

    f_i = np.arange(F)[:, None, None, None, None]
    c_i = np.arange(C)[None, :, None, None, None]
    j_i = np.arange(NCH)[None, None, :, None, None]
    dd_i = np.arange(8)[None, None, None, :, None]
    m_i = np.arange(NM)[None, None, None, None, :]
    l = (8 * m_i + dd_i) + 79 - 21 * j_i - c_i               # (F,C,NCH,8,NM)
    ok = (l >= 0) & (l < L)
    g = delayed[:, f_i, np.clip(l, 0, L - 1)]                # (E,F,C,NCH,8,NM)
    g = np.where(ok[None], g, 0.0)
    # -> [(f,c), j, m, dd, e]
    W = g.transpose(1, 2, 3, 5, 4, 0).reshape(KP, NCH, NM, 8 * E)
    return np.ascontiguousarray(W).astype(BF16)


def _build_xsc(x: np.ndarray) -> np.ndarray:
    """Xsc[core, (f,c), :] = block-major concat of [NCH, n_blk] chunk rows:
    chunk j of column col = 1366*b_local + n reads x[b, 24n + 21j + c - 39, f]."""
    need = Q * (NCOLB - 1) + 21 * (NCH - 1) + C
    xpad = np.zeros((B, PADL + need, F), dtype=np.float32)
    xpad[:, PADL:PADL + S, :] = x
    sb, st, sf = xpad.strides
    # V[b, (f,c), j, n] = xpad[b, 24n + 21j + c, f]
    V = np.lib.stride_tricks.as_strided(
        xpad, shape=(B, F, C, NCH, NCOLB), strides=(sb, sf, st, 21 * st, Q * st)
    )
    V16 = V.astype(BF16).reshape(B, KP, NCH, NCOLB)
    Xc = np.empty((NCORES, KP, NCH, CTOT), dtype=BF16)
    for b in range(B):
        core, i = divmod(b, BPC)
        Xc[core, :, :, i * NCOLB:(i + 1) * NCOLB] = V16[b]
    out = np.empty((NCORES, KP, NCH * CTOT), dtype=BF16)
    for off, n in zip(BOFF, BLOCKS):
        out[:, :, NCH * off:NCH * (off + n)] = (
            Xc[:, :, :, off:off + n].reshape(NCORES, KP, NCH * n)
        )
    return np.ascontiguousarray(out)


def _build_program():
    f32 = mybir.dt.float32
    bf16 = mybir.dt.bfloat16
    nc = bacc.Bacc("TRN2", target_bir_lowering=False, debug=False)
    xsc = nc.dram_tensor("xsc", [KP, NCH * CTOT], bf16, kind="ExternalInput")
    w = nc.dram_tensor("w", [KP, NCH, NM, 128], bf16, kind="ExternalInput")
    osc = nc.dram_tensor("osc", [128, NM * CTOT], bf16, kind="ExternalOutput")

    with TileContext(nc) as tc:
        with (
            tc.tile_pool(name="wp", bufs=1) as wp,
            tc.tile_pool(name="xp", bufs=6) as xp,
            tc.tile_pool(name="pp", bufs=8, space="PSUM") as pp,
            tc.tile_pool(name="op", bufs=6) as op,
        ):
            wt = wp.tile([KP, NCH * NM * 128], bf16)     # [(f,c), (j, m, col)]
            wr = w.rearrange("k j m n -> k (j m n)")
            xtiles = {}

            def issue_w(j):
                sl = slice(j * NM * 128, (j + 1) * NM * 128)
                nc.sync.dma_start(out=wt[:, sl], in_=wr[:, sl])

            def issue_x(blk):
                off, n = BOFF[blk], BLOCKS[blk]
                xt = xp.tile([KP, NCH * n], bf16, tag="xt", name=f"xt_{blk}")
                nc.sync.dma_start(out=xt, in_=xsc[:, NCH * off:NCH * (off + n)])
                xtiles[blk] = xt

            def wslice(j, m):
                return wt[:, (j * NM + m) * 128:(j * NM + m + 1) * 128]

            # DMA order: first matmul gates on W(j0)+X0 (~0.25MB); later
            # pieces and x blocks stream in behind it.
            issue_w(0); issue_x(0)
            issue_w(1); issue_x(1)
            issue_w(2); issue_x(2)
            issue_w(3); issue_w(4)

            for blk in range(NBLK):
                off, n = BOFF[blk], BLOCKS[blk]
                if blk + 3 < NBLK:
                    issue_x(blk + 3)
                xt = xtiles.pop(blk)
                pss = [
                    pp.tile([128, n], f32, tag="ps", name=f"ps_{blk}_{m}")
                    for m in range(NM)
                ]
                ot = op.tile([128, NM * n], bf16, tag="ot", name=f"ot_{blk}")

                def evac(m, n=n, pss=pss, ot=ot):
                    nc.vector.tensor_copy(out=ot[:, m * n:(m + 1) * n], in_=pss[m])

                if blk < N_JOUTER:
                    # j-outer: each arriving W(j) piece feeds all 3 M-tiles.
                    for j in range(NCH):
                        for m in range(NM):
                            nc.tensor.matmul(
                                pss[m],
                                wslice(j, m),
                                xt[:, j * n:(j + 1) * n],
                                start=(j == 0),
                                stop=(j == NCH - 1),
                                skip_group_check=True,
                            )
                    for m in range(NM):
                        evac(m)
                else:
                    # m-outer: M-tiles complete one after another, so PSUM
                    # evacuation staggers across the block.
                    for m in range(NM):
                        for j in range(NCH):
                            nc.tensor.matmul(
                                pss[m],
                                wslice(j, m),
                                xt[:, j * n:(j + 1) * n],
                                start=(j == 0),
                                stop=(j == NCH - 1),
                            )
                        evac(m)
                nc.sync.dma_start(
                    out=osc[:, NM * off:NM * (off + n)], in_=ot
                )
    nc.compile()   # bacc passes: split multi-waits (HW allows 1 wait/inst), DCE, reg alloc
    return nc


def kernel(x: np.ndarray, templates: np.ndarray, onset_delays: np.ndarray) -> np.ndarray:
    global LAST_RESULT
    x = np.ascontiguousarray(x, dtype=np.float32)
    templates = np.asarray(templates, dtype=np.float32)
    onset_delays = np.asarray(onset_delays, dtype=np.float32)

    W = _build_weights(templates, onset_delays)
    Xsc = _build_xsc(x)                                   # (NCORES, KP, NCH*CTOT)

    nc = _build_program()
    in_maps = [{"xsc": Xsc[c], "w": W} for c in range(NCORES)]
    res = run_bass_kernel_spmd(nc, in_maps, core_ids=list(range(NCORES)))
    LAST_RESULT = res

    osc = np.stack([r["osc"] for r in res.results], axis=0)   # (NCORES,128,NM*CTOT)
    osc = osc.astype(np.float32)
    O = np.empty((NCORES, 128, NM, CTOT), dtype=np.float32)
    for off, n in zip(BOFF, BLOCKS):
        O[:, :, :, off:off + n] = (
            osc[:, :, NM * off:NM * (off + n)].reshape(NCORES, 128, NM, n)
        )
    o = O.reshape(NCORES, 8, E, NM, BPC, NCOLB)           # c, dd, e, m, b, n
    o = o.transpose(0, 4, 5, 3, 1, 2)                      # c, b, n, m, dd, e
    o = np.ascontiguousarray(o).reshape(B, NCOLB * Q, E)[:, :S, :]
    o = np.ascontiguousarray(o)
    o[:, S - 1, :] = 0.0                                   # reference zero-pads last column
    return o
